# revision 1
# baseline (speedup 1.0000x reference)
"""Trainium2 Bass kernel for nn_LocalizationLoss.

Loss (see reference):
  p = out[:,:,0]; t = tgt[:,:,0] in {0,1}; mask = t
  bce  = -mean(t*ln(p) + (1-t)*ln(1-p))
  trick= out * t[...,None]
  CE over slot axis (dim 1) of trick[:,:,4:7] with targets tgt[:,:,4]
  Lx   = mean((trick_x - tx)^2), Ly likewise
  Lwh  = mean((t*sqrt(ow) - sqrt(tw))^2)
  loss = 5*(Lx+Ly+2*Lwh) + bce + 0.5*(1-bce) + 3*ce

Device computes, per core (batch-sharded), per-partition partial sums:
  S_bce  = sum ln|p + t - 1|            (== t*ln p + (1-t)*ln(1-p))
  S_sqxy = sum (t*ox-tx)^2 + (t*oy-ty)^2
  S_mwtw = sum (t*ow + tw)
  S_ts2  = sum t*2*sqrt(ow*tw)      [sqrt via exp(0.5*ln(m)+ln2), one ACT set]
  S_lse  = sum_j ln sum_i exp(t_i*o_i[4+j])
  S_seli = sum_j (tgt_j==i) * t_i*o_i[4+j]   for i in 0,1,2
Host: Swh = S_mwtw - S_ts2  (since (t*sqrt(ow)-sqrt(tw))^2
      == t*ow - 2*t*sqrt(ow*tw) + tw for t in {0,1})
      ce*3B = S_lse - (S_sel0+S_sel1+S_sel2)
      loss = 0.5 + (5*S_sqxy + 10*Swh - 0.5*S_bce + 3*ce*3B) / (3B)
"""

import numpy as np

import concourse.bass as bass
import concourse.bacc as bacc
import concourse.mybir as mybir
from concourse.tile import TileContext
from concourse.bass_utils import run_bass_kernel_spmd

# Force the ACT table pass to use only natural_log_exp_and_others (it holds
# every func this kernel needs: ln/exp/square/abs/copy/identity). The default
# greedy per-func set choice thrashes between sets, costing a ~1.3us
# ACT_TABLE_LOAD each time. Blank the other sets, keep dict order so
# act_func_set_id indices stay aligned with act_info.json.
import concourse.hw_specs as _hw_specs
if not hasattr(_hw_specs, "_orig_get_activation_tables"):
    _hw_specs._orig_get_activation_tables = _hw_specs.get_activation_tables

    def _only_ln_exp_tables(module_arch):
        tabs = _hw_specs._orig_get_activation_tables(module_arch)
        return {
            name: (funcs if name == "natural_log_exp_and_others" else set())
            for name, funcs in tabs.items()
        }

    _hw_specs.get_activation_tables = _only_ln_exp_tables
    import concourse.bacc as _bacc_mod
    if hasattr(_bacc_mod, "get_activation_tables"):
        _bacc_mod.get_activation_tables = _only_ln_exp_tables

F32 = mybir.dt.float32
BF16 = mybir.dt.bfloat16
ALU = mybir.AluOpType
ACT = mybir.ActivationFunctionType
LN2 = 0.6931471805599453

P = 128          # SBUF partitions
N_CORES = 8
B_FULL = 1_048_576

# per-chunk partial-sum column layout
(COL_BCE, COL_SQXY, COL_MWTW, COL_TS2, COL_LSE,
 COL_SEL0, COL_SEL1, COL_SEL2) = range(8)
NCOL_PER_CHUNK = 8

# rows-per-partition chunk sizes (each divisible by 3). A small first chunk
# hides the initial DMA latency; later chunks are big to amortize overheads.
CHUNKS_FULL = (192, 384, 768, 864, 864)     # sums to 3072 = rpp for full size


def build_kernel(nb: int, chunks) -> bass.Bass:
    """Build the per-core Bass program for nb batch elements (ROWS=nb*3)."""
    rows = nb * 3
    assert rows % P == 0
    rpp = rows // P                 # rows per partition
    chunks = list(chunks)
    assert sum(chunks) == rpp, (sum(chunks), rpp)
    assert all(r % 3 == 0 for r in chunks)
    n_chunks = len(chunks)
    ncols = NCOL_PER_CHUNK * n_chunks

    nc = bacc.Bacc()

    # Const [128,1] APs for activation bias values (non-Copy funcs need AP
    # bias; only 0.0/1.0 are pre-registered by Bass.__init__).
    for val in (-1.0, -0.001, 0.001, LN2):
        ctile = nc.alloc_sbuf_tensor(f"const-f32-{val}", [128, 1], F32)
        nc.gpsimd.memset(ctile.ap(), val)
        nc.const_aps.aps[(F32, val)] = ctile.ap()
    nc.all_engine_barrier()

    out_hbm = nc.declare_dram_parameter("output", [rows * 7], F32, isOutput=False)
    tgt_hbm = nc.declare_dram_parameter("target", [rows * 5], F32, isOutput=False)
    res_hbm = nc.declare_dram_parameter("res", [P, ncols], F32, isOutput=True)

    out_v = out_hbm[:].rearrange("(p n) -> p n", p=P)   # [128, rpp*7]
    tgt_v = tgt_hbm[:].rearrange("(p n) -> p n", p=P)   # [128, rpp*5]

    with TileContext(nc) as tc:
        with (
            tc.tile_pool(name="io", bufs=2) as io_pool,
            tc.tile_pool(name="mid", bufs=2) as mid_pool,
            tc.tile_pool(name="accp", bufs=1) as acc_pool,
        ):
            cols = acc_pool.tile([P, ncols], F32)
            row0 = 0
            for c, R in enumerate(chunks):
                cb = c * NCOL_PER_CHUNK
                G = R // 3

                ot = io_pool.tile([P, R * 7], BF16, tag="ot")
                tt = io_pool.tile([P, R * 5], BF16, tag="tt")
                # gpsimd (SWDGE) DMA casts f32 DRAM -> bf16 SBUF for free
                nc.gpsimd.dma_start(out=ot[:, :], in_=out_v[:, row0 * 7:(row0 + R) * 7])
                nc.gpsimd.dma_start(out=tt[:, :], in_=tgt_v[:, row0 * 5:(row0 + R) * 5])
                row0 += R

                o3 = ot[:, :].rearrange("p (r c) -> p r c", c=7)    # [128,R,7]
                t5 = tt[:, :].rearrange("p (r c) -> p r c", c=5)    # [128,R,5]

                p_ch = o3[:, :, 0]
                ow = o3[:, :, 3]
                t_ch = t5[:, :, 0]
                tw = t5[:, :, 3]
                # tgt channel in (j,g) iteration order: offset 15g+5j+4
                tgt_jg = tt[:, :].rearrange("p (g j c) -> p j g c", j=3, c=5)[:, :, :, 4]

                # ---- scratch tiles (per chunk) ----
                # Mxyw: planes x,y,w (masked o-ch 1..3), each dense [R]
                # Mlog: 9 planes (i,j) of masked logits, each dense [G]
                Mxyw = mid_pool.tile([P, R * 3], BF16, tag="Mxyw")
                Mlog = mid_pool.tile([P, R * 3], BF16, tag="Mlog")
                E = mid_pool.tile([P, R * 3], BF16, tag="E")     # exp(Mlog), same planes
                S = mid_pool.tile([P, R], BF16, tag="S")         # sum_i E, (j,g) dense
                qs = mid_pool.tile([P, R], F32, tag="qs")        # p + t (f32: |1+p-1|)
                exy = mid_pool.tile([P, R * 2], BF16, tag="exy")
                m = mid_pool.tile([P, R], F32, tag="m")          # f32: ln->exp roundtrip
                s2b = mid_pool.tile([P, R], BF16, tag="s2b")     # 2*sqrt(m) in bf16
                tgtd = mid_pool.tile([P, R], BF16, tag="tgtd")   # tgt, (j,g) dense
                junkv = mid_pool.tile([P, R], BF16, tag="junkv")
                junkv2 = mid_pool.tile([P, R], BF16, tag="junkv2")
                junka = mid_pool.tile([P, R], BF16, tag="junka")

                Mxyw_pl = Mxyw[:, :].rearrange("p (c r) -> p c r", c=3)   # [128,3,R]
                # Mlog plane (i,j) at offset (3i+j)*G; views:
                Mlog_ijg = Mlog[:, :].rearrange("p (i j g) -> p i j g", i=3, j=3)
                # for S adds / sel: fixed i -> [128, 3(j), G] dense runs
                E_ijg = E[:, :].rearrange("p (i j g) -> p i j g", i=3, j=3)
                S_jg = S[:, :].rearrange("p (j g) -> p j g", j=3)
                exy_pl = exy[:, :].rearrange("p (c r) -> p c r", c=2)

                # ---- V1a: masked x,y,w planes: iterate (c,r) ----
                o_xyw = ot[:, :].rearrange("p (r c) -> p c r", c=7)[:, 1:4, :]
                t_b3 = t5[:, :, 0:1].broadcast_to([P, R, 3]).rearrange("p r c -> p c r")
                nc.vector.tensor_tensor(Mxyw_pl, o_xyw, t_b3, ALU.mult)

                # ---- V1b: masked logit planes (i,j): iterate (i,j,g) ----
                o_lg = ot[:, :].rearrange("p (g i c) -> p i c g", i=3, c=7)[:, :, 4:7, :]
                t_bl = (
                    tt[:, :].rearrange("p (g i c) -> p i c g", i=3, c=5)[:, :, 0:1, :]
                    .broadcast_to([P, 3, 3, G])
                )
                nc.vector.tensor_tensor(Mlog_ijg, o_lg, t_bl, ALU.mult)

                # ---- BCE: q = |p + t - 1| (f32 add of bf16 inputs);
                # clamp tiny q (bf16(p)==1.0 cliff) via ln(max(q,1e-3)) ----
                nc.vector.tensor_tensor(qs[:, :], p_ch, t_ch, ALU.add)
                nc.scalar.activation(qs[:, :], qs[:, :], ACT.Abs, bias=-1.0, scale=1.0)
                nc.scalar.activation(qs[:, :], qs[:, :], ACT.Relu, bias=-0.001, scale=1.0)
                nc.scalar.activation(
                    qs[:, :], qs[:, :], ACT.Ln, bias=0.001, scale=1.0,
                    accum_out=cols[:, cb + COL_BCE:cb + COL_BCE + 1],
                )

                # ---- x/y MSE: exy = Mxy - txy ; col += sum square ----
                t_xy = t5[:, :, 1:3].rearrange("p r c -> p c r")    # [128,2,R]
                nc.vector.tensor_tensor(exy_pl, Mxyw_pl[:, 0:2, :], t_xy, ALU.subtract)
                nc.scalar.activation(
                    exy[:, :], exy[:, :], ACT.Square,
                    accum_out=cols[:, cb + COL_SQXY:cb + COL_SQXY + 1],
                )

                # ---- wh: m = ow*tw; s2 = 2*sqrt(m) = exp(0.5*ln(m)+ln2) ----
                nc.vector.tensor_tensor(m[:, :], ow, tw, ALU.mult)
                nc.scalar.activation(m[:, :], m[:, :], ACT.Ln)
                nc.scalar.activation(s2b[:, :], m[:, :], ACT.Exp, bias=LN2, scale=0.5)
                # ts2 = t * s2 (dense product), summed by ACT copy-accum
                nc.vector.tensor_tensor(junkv[:, :], s2b[:, :], t_ch, ALU.mult)
                nc.scalar.activation(
                    junka[:, :], junkv[:, :], ACT.Copy,
                    accum_out=cols[:, cb + COL_TS2:cb + COL_TS2 + 1],
                )
                # mwtw = t*ow + tw (w plane is dense), summed by ACT copy-accum
                nc.vector.tensor_tensor(junkv2[:, :], Mxyw_pl[:, 2, :], tw, ALU.add)
                nc.scalar.activation(
                    junka[:, :], junkv2[:, :], ACT.Copy,
                    accum_out=cols[:, cb + COL_MWTW:cb + COL_MWTW + 1],
                )

                # ---- CE: E = exp(Mlog) (fully dense); S_j = sum_i E ----
                nc.scalar.activation(E[:, :], Mlog[:, :], ACT.Exp)
                nc.vector.tensor_tensor(S_jg, E_ijg[:, 0], E_ijg[:, 1], ALU.add)
                nc.vector.tensor_tensor(S_jg, S_jg, E_ijg[:, 2], ALU.add)
                nc.scalar.activation(
                    S[:, :], S[:, :], ACT.Ln,
                    accum_out=cols[:, cb + COL_LSE:cb + COL_LSE + 1],
                )

                # ---- CE select: tgt staged dense, then 3 dense fused ops ----
                nc.vector.tensor_scalar(tgtd[:, :], tgt_jg, 1.0, None, ALU.mult)
                tgtd_jg = tgtd[:, :].rearrange("p (j g) -> p j g", j=3)
                for i in range(3):
                    nc.vector.scalar_tensor_tensor(
                        junkv2[:, :].rearrange("p (j g) -> p j g", j=3),
                        tgtd_jg, float(i), Mlog_ijg[:, i], ALU.is_equal, ALU.mult,
                        accum_out=cols[:, cb + COL_SEL0 + i:cb + COL_SEL0 + i + 1],
                    )

            nc.sync.dma_start(out=res_hbm[:, :], in_=cols[:, :])

    nc.compile()
    return nc


def combine_results(res_list, n_chunks: int, b_total: int) -> np.float32:
    """Host-side combine of per-core [128, ncols] partial sums."""
    acc = np.zeros(NCOL_PER_CHUNK, dtype=np.float64)
    for res in res_list:
        r = np.asarray(res).astype(np.float64).reshape(P, n_chunks, NCOL_PER_CHUNK)
        acc += r.sum(axis=(0, 1))
    s_bce = acc[COL_BCE]
    s_sqxy = acc[COL_SQXY]
    s_wh = acc[COL_MWTW] - acc[COL_TS2]
    s_ce = acc[COL_LSE] - (acc[COL_SEL0] + acc[COL_SEL1] + acc[COL_SEL2])
    denom = 3.0 * b_total
    loss = 0.5 + (5.0 * s_sqxy + 10.0 * s_wh - 0.5 * s_bce + 3.0 * s_ce) / denom
    return np.float32(loss)


_CACHED = {}


def _chunks_for(nb: int):
    rpp = nb * 3 // P
    if rpp == 3072:
        return CHUNKS_FULL
    # fallback: split into up to 4 equal chunks divisible by 3
    for n in (4, 2, 1):
        if rpp % n == 0 and (rpp // n) % 3 == 0:
            return (rpp // n,) * n
    return (rpp,)


def _get_nc(nb: int):
    chunks = _chunks_for(nb)
    key = (nb, chunks)
    if key not in _CACHED:
        _CACHED[key] = (build_kernel(nb, chunks), len(chunks))
    return _CACHED[key]


def run_on_cores(output: np.ndarray, target: np.ndarray, trace: bool = False):
    """Shard along batch, run on 8 cores, return (res_list, n_chunks, results)."""
    b = output.shape[0]
    nb = b // N_CORES
    nc, n_chunks = _get_nc(nb)
    in_maps = []
    for k in range(N_CORES):
        o = np.ascontiguousarray(output[k * nb:(k + 1) * nb]).reshape(-1)
        t = np.ascontiguousarray(target[k * nb:(k + 1) * nb]).reshape(-1)
        in_maps.append({"output": o, "target": t})
    results = run_bass_kernel_spmd(
        nc, in_maps, core_ids=list(range(N_CORES)), trace=trace
    )
    res_list = [r["res"] for r in results.results]
    return res_list, n_chunks, results


def kernel(output: np.ndarray, target: np.ndarray) -> np.ndarray:
    output = np.asarray(output, dtype=np.float32)
    target = np.asarray(target, dtype=np.float32)
    b = output.shape[0]
    res_list, n_chunks, _ = run_on_cores(output, target)
    return combine_results(res_list, n_chunks=n_chunks, b_total=b)

